# revision 20
# baseline (speedup 1.0000x reference)
"""Trainium2 Bass kernel for the Combined MIL Feature-Attention MultiBag model.

Math (per bag b of B=256; T=500 instances, M=512 features, L=128 attn dim):
    V = tanh(H @ Wv.T + bv);  U = sigmoid(H @ Wu.T + bu)
    A_pre = (V*U) @ Ww.T + bw                      -> [T] logits
    a = softmax(A_pre)                             -> [T]
    Mfeat = a @ H                                  -> [M]
    out = Mfeat @ Wc.T + bc * sum(a)  (sum(a) == 1.0)
Returns (out [B,2], A_sftmx [B,T], A_pre [B,1,T]).

Strategy: pure data-parallel over 8 NeuronCores (32 bags each). The host ships
H twice in fp16 -- natural [T,M] layout (pooling contracts T) and transposed
[M,T] layout (V/U matmuls contract M) -- so the kernel needs no on-chip
transposes of H while HBM traffic stays equal to reading fp32 H once.
Sigmoid is computed as 0.5+0.5*tanh(x/2) (0.5 folded into Ww) so ScalarE only
needs the exp_and_others LUT set (tanh+exp): one table load, no thrash.
Softmax is batched across groups of 8 bags on partitions; its normalization
uses Exp's accum_out for the denominator. bias_rel == sum(softmax) == 1.0 is
used directly (error ~1e-9).
"""

import sys

if "/opt/trn_rl_repo" not in sys.path:
    sys.path.insert(0, "/opt/trn_rl_repo")

from contextlib import ExitStack

import numpy as np

import concourse.bass as bass
import concourse.tile as tile
from concourse import bacc, mybir

B, T, M, L = 256, 500, 512, 128
NCORES = 8
BSH = B // NCORES  # bags per core
TP = 125           # T-chunk (partition) size for natural layout; 4 chunks
NTC = T // TP      # 4
NMC = M // 128     # 4 M-chunks for transposed layout

F16 = mybir.dt.float16
F32 = mybir.dt.float32
AX = mybir.AxisListType
OP = mybir.AluOpType
AF = mybir.ActivationFunctionType


def build_bass(bsh=BSH, gb=8, stage=5):
    """Build the per-core Bass program. bsh bags per core, softmax-batched in
    groups of gb. stage<4 builds a prefix of the pipeline (debug bisection):
    1=logits only, 2=+softmax, 3=+a-transpose, 4=+pooling, 5=full."""
    assert bsh % gb == 0
    ng = bsh // gb
    nc = bacc.Bacc("TRN2", target_bir_lowering=False, debug=False)

    h_nat = nc.dram_tensor("h_nat", [bsh, T, M], F16, kind="ExternalInput").ap()
    h_tr = nc.dram_tensor("h_tr", [bsh, M, T], F16, kind="ExternalInput").ap()
    wvut = nc.dram_tensor("wvut", [M, 2 * L], F16, kind="ExternalInput").ap()
    wwt = nc.dram_tensor("wwt", [L, 1], F16, kind="ExternalInput").ap()
    bv_i = nc.dram_tensor("bv_i", [L, 1], F32, kind="ExternalInput").ap()
    buh_i = nc.dram_tensor("buh_i", [L, 1], F32, kind="ExternalInput").ap()
    bw_i = nc.dram_tensor("bw_i", [gb, 1], F32, kind="ExternalInput").ap()
    wc_i = nc.dram_tensor("wc_i", [bsh, 2, M], F32, kind="ExternalInput").ap()
    bc_i = nc.dram_tensor("bc_i", [bsh, 2], F32, kind="ExternalInput").ap()
    id_i = nc.dram_tensor("id_i", [gb, gb], F16, kind="ExternalInput").ap()

    out_o = nc.dram_tensor("out_o", [bsh, 2], F32, kind="ExternalOutput").ap()
    asf_o = nc.dram_tensor("asf_o", [bsh, T], F32, kind="ExternalOutput").ap()
    apre_o = nc.dram_tensor("apre_o", [bsh, T], F32, kind="ExternalOutput").ap()

    with tile.TileContext(nc) as tc, ExitStack() as ctx:
        consts = ctx.enter_context(tc.tile_pool(name="consts", bufs=1))
        htr_p = ctx.enter_context(tc.tile_pool(name="htr", bufs=10))
        hnat_p = ctx.enter_context(tc.tile_pool(name="hnat", bufs=10))
        act_p = ctx.enter_context(tc.tile_pool(name="act", bufs=5))
        grp_p = ctx.enter_context(tc.tile_pool(name="grp", bufs=3))
        mf_p = ctx.enter_context(tc.tile_pool(name="mf", bufs=1))
        ps_v = ctx.enter_context(tc.tile_pool(name="ps_v", bufs=2, space="PSUM"))
        ps_u = ctx.enter_context(tc.tile_pool(name="ps_u", bufs=2, space="PSUM"))
        ps_a = ctx.enter_context(tc.tile_pool(name="ps_a", bufs=2, space="PSUM"))
        ps_mf = ctx.enter_context(tc.tile_pool(name="ps_mf", bufs=1, space="PSUM"))
        ps_at = ctx.enter_context(tc.tile_pool(name="ps_at", bufs=1, space="PSUM"))

        # --- constants ---
        wvut_t = consts.tile([128, NMC, 2 * L], F16)
        nc.sync.dma_start(wvut_t, wvut.rearrange("(c p) l -> p c l", p=128))
        wwt_t = consts.tile([L, 1], F16)
        nc.sync.dma_start(wwt_t, wwt)
        bv_t = consts.tile([L, 1], F32)
        nc.sync.dma_start(bv_t, bv_i)
        buh_t = consts.tile([L, 1], F32)
        nc.sync.dma_start(buh_t, buh_i)
        bw_t = consts.tile([gb, 1], F32)
        nc.sync.dma_start(bw_t, bw_i)
        wc_t = consts.tile([bsh, 2, M], F32)
        nc.sync.dma_start(wc_t, wc_i)
        bc_t = consts.tile([bsh, 2], F32)
        nc.sync.dma_start(bc_t, bc_i)
        id_t = consts.tile([gb, gb], F16)
        nc.sync.dma_start(id_t, id_i)

        # ScalarE warmup. The ACT descriptor takes at most 2 sync waits and
        # the first use of a LUT set spends one on the table load, so: first
        # observe the const DMAs with plain copies (1 wait each), then
        # trigger the exp_and_others table load on inputs ScalarE itself
        # produced (no new waits). Exp first so walrus picks the set that
        # also contains tanh.
        warm = consts.tile([L, 1], F32)
        warm2 = consts.tile([L, 1], F32)
        nc.scalar.copy(warm, bv_t)
        nc.scalar.copy(warm2, buh_t)
        nc.scalar.activation(warm, warm2, AF.Exp)
        nc.scalar.activation(warm, warm2, AF.Tanh)

        mfeat_all = mf_p.tile([bsh, M], F32)

        for g in range(ng):
            apre_g = grp_p.tile([gb, T], F32, tag="apre_g")
            for j in range(gb):
                b = g * gb + j
                htr_t = htr_p.tile([128, NMC, T], F16, tag="htr")
                # alternate the two HWDGE rings (SP / ACT) so consecutive
                # 512 KB loads overlap instead of serializing on one ring
                ldeng = nc.sync if b % 2 == 0 else nc.scalar
                ldeng.dma_start(htr_t, h_tr[b].rearrange("(c p) t -> p c t", p=128))
                vps = ps_v.tile([L, T], F32, tag="v")
                ups = ps_u.tile([L, T], F32, tag="u")
                for c in range(NMC):
                    nc.tensor.matmul(
                        vps, wvut_t[:, c, 0:L], htr_t[:, c, :],
                        start=(c == 0), stop=(c == NMC - 1),
                    )
                for c in range(NMC):
                    nc.tensor.matmul(
                        ups, wvut_t[:, c, L : 2 * L], htr_t[:, c, :],
                        start=(c == 0), stop=(c == NMC - 1),
                    )
                tv = act_p.tile([L, T], F16, tag="tv")
                nc.scalar.activation(tv, vps, AF.Tanh, bias=bv_t, scale=1.0)
                tu = act_p.tile([L, T], F16, tag="tu")
                nc.scalar.activation(tu, ups, AF.Tanh, bias=buh_t, scale=0.5)
                # VU' = (tu + 1) * tv  == 2 * V * sigmoid(U); the 1/2 lives in wwt
                vu = act_p.tile([L, T], F16, tag="vu")
                nc.vector.scalar_tensor_tensor(
                    vu, tu, 1.0, tv, op0=OP.add, op1=OP.mult
                )
                aps = ps_a.tile([1, T], F32, tag="apre")
                nc.tensor.matmul(aps, wwt_t, vu, start=True, stop=True)
                # bounce PSUM->SBUF (DMA can't read PSUM), then DMA shifts
                # the row onto partition j of the group tile
                aprow = act_p.tile([1, T], F32, tag="aprow")
                nc.vector.tensor_copy(aprow, aps)
                nc.gpsimd.dma_start(apre_g[j : j + 1, :], aprow)

            # --- group softmax over T (free dim), 8 bags on partitions ---
            apre_out = grp_p.tile([gb, T], F32, tag="apre_out")
            nc.vector.tensor_scalar_add(apre_out, apre_g, bw_t)
            nc.gpsimd.dma_start(apre_o[g * gb : (g + 1) * gb, :], apre_out)
            if stage < 2:
                continue
            negmax = grp_p.tile([gb, 1], F32, tag="negmax")
            nc.vector.tensor_reduce(
                negmax, apre_g, axis=AX.X, op=OP.max, negate=True
            )
            aexp = grp_p.tile([gb, T], F32, tag="aexp")
            zsum = grp_p.tile([gb, 1], F32, tag="zsum")
            nc.scalar.activation(
                aexp, apre_g, AF.Exp, bias=negmax, scale=1.0, accum_out=zsum
            )
            rz = grp_p.tile([gb, 1], F32, tag="rz")
            nc.vector.reciprocal(rz, zsum)
            asf_t = grp_p.tile([gb, T], F32, tag="asf")
            nc.vector.tensor_scalar_mul(asf_t, aexp, rz)
            nc.gpsimd.dma_start(asf_o[g * gb : (g + 1) * gb, :], asf_t)
            if stage < 3:
                continue
            an16 = grp_p.tile([gb, T], F16, tag="an16")
            nc.vector.tensor_scalar_mul(an16, aexp, rz)
            # aT[p, c, j] = a_norm[j, c*TP + p] via PE transpose of each T-chunk
            atps = ps_at.tile([TP, NTC, gb], F16, tag="atps")
            for c in range(NTC):
                nc.tensor.transpose(
                    atps[:, c, :], an16[:, c * TP : (c + 1) * TP], id_t
                )
            at_sb = grp_p.tile([TP, NTC, gb], F16, tag="at_sb")
            nc.vector.tensor_copy(at_sb, atps)
            if stage < 4:
                continue

            # --- pooling: Mfeat_b = sum_t a[t] * H[t, :] ---
            for j in range(gb):
                b = g * gb + j
                hnat_t = hnat_p.tile([TP, NTC, M], F16, tag="hnat")
                ldeng = nc.scalar if b % 2 == 0 else nc.sync
                ldeng.dma_start(
                    hnat_t, h_nat[b].rearrange("(c p) m -> p c m", p=TP)
                )
                mfps = ps_mf.tile([1, M], F32, tag="mf")
                for c in range(NTC):
                    nc.tensor.matmul(
                        mfps, at_sb[:, c, j : j + 1], hnat_t[:, c, :],
                        start=(c == 0), stop=(c == NTC - 1),
                    )
                mfrow = act_p.tile([1, M], F32, tag="mfrow")
                nc.vector.tensor_copy(mfrow, mfps)
                nc.gpsimd.dma_start(mfeat_all[b : b + 1, :], mfrow)

        # --- classifier: out = Mfeat @ Wc.T + bc ---
        if stage >= 5:
            prod = grp_p.tile([bsh, 2, M], F32, tag="prod")
            out_sb = grp_p.tile([bsh, 2], F32, tag="out_sb")
            for k in range(2):
                nc.vector.tensor_mul(prod[:, k, :], mfeat_all, wc_t[:, k, :])
            nc.vector.tensor_reduce(out_sb, prod, axis=AX.X, op=OP.add)
            out_fin = grp_p.tile([bsh, 2], F32, tag="out_fin")
            nc.vector.tensor_add(out_fin, out_sb, bc_t)
            nc.gpsimd.dma_start(out_o, out_fin)

    # Bacc legalizes sync waits (>=2 waits get split onto event-semaphore
    # instructions; matmul waits move to ldweights) -- required by walrus.
    nc.compile()
    return nc


def make_in_maps(H_cat, Wv, bv, Wu, bu, Ww, bw, Wc, bc, n_cores=NCORES, bsh=BSH, gb=8):
    """Host-side prep: shard bags, pre-transpose/cast H, pack weights."""
    H = np.ascontiguousarray(np.asarray(H_cat, dtype=np.float32))
    h16 = H.astype(np.float16)
    ht16 = np.ascontiguousarray(h16.transpose(0, 2, 1))
    wvut = np.ascontiguousarray(
        np.concatenate(
            [np.asarray(Wv, np.float32).T, np.asarray(Wu, np.float32).T], axis=1
        )
    ).astype(np.float16)
    wwt = (np.asarray(Ww, np.float32)[0] / 2.0).reshape(L, 1).astype(np.float16)
    bv_a = np.asarray(bv, np.float32).reshape(L, 1)
    buh = (np.asarray(bu, np.float32) / 2.0).reshape(L, 1)
    bw_a = np.full((gb, 1), np.float32(np.asarray(bw, np.float32)[0]), np.float32)
    wc_rep = np.ascontiguousarray(
        np.broadcast_to(np.asarray(Wc, np.float32)[None], (bsh, 2, M))
    )
    bc_rep = np.ascontiguousarray(
        np.broadcast_to(np.asarray(bc, np.float32)[None], (bsh, 2))
    )
    id8 = np.eye(gb, dtype=np.float16)
    in_maps = []
    for i in range(n_cores):
        sl = slice(i * bsh, (i + 1) * bsh)
        in_maps.append(
            {
                "h_nat": h16[sl],
                "h_tr": ht16[sl],
                "wvut": wvut,
                "wwt": wwt,
                "bv_i": bv_a,
                "buh_i": buh,
                "bw_i": bw_a,
                "wc_i": wc_rep,
                "bc_i": bc_rep,
                "id_i": id8,
            }
        )
    return in_maps


_NC_CACHE = {}


def get_nc(bsh=BSH, gb=8):
    key = (bsh, gb)
    if key not in _NC_CACHE:
        _NC_CACHE[key] = build_bass(bsh, gb)
    return _NC_CACHE[key]


def run(inputs, trace=False, **kwargs):
    from concourse.bass_utils import run_bass_kernel_spmd

    nc = get_nc()
    in_maps = make_in_maps(**inputs)
    res = run_bass_kernel_spmd(
        nc, in_maps, core_ids=list(range(NCORES)), trace=trace, **kwargs
    )
    outs = res.results
    out = np.concatenate([r["out_o"] for r in outs], axis=0)
    asf = np.concatenate([r["asf_o"] for r in outs], axis=0)
    apre = np.concatenate([r["apre_o"] for r in outs], axis=0).reshape(B, 1, T)
    return (out, asf, apre), res


def kernel(**inputs):
    (out, asf, apre), _ = run(inputs, trace=False)
    return out, asf, apre


# revision 21
# speedup vs baseline: 1.0534x; 1.0534x over previous
"""Trainium2 Bass kernel for the Combined MIL Feature-Attention MultiBag model.

Math (per bag b of B=256; T=500 instances, M=512 features, L=128 attn dim):
    V = tanh(H @ Wv.T + bv);  U = sigmoid(H @ Wu.T + bu)
    A_pre = (V*U) @ Ww.T + bw                      -> [T] logits
    a = softmax(A_pre)                             -> [T]
    Mfeat = a @ H                                  -> [M]
    out = Mfeat @ Wc.T + bc * sum(a)  (sum(a) == 1.0)
Returns (out [B,2], A_sftmx [B,T], A_pre [B,1,T]).

Strategy: pure data-parallel over 8 NeuronCores (32 bags each). The host ships
H twice in fp16 -- natural [T,M] layout (pooling contracts T) and transposed
[M,T] layout (V/U matmuls contract M) -- so the kernel needs no on-chip
transposes of H while HBM traffic stays equal to reading fp32 H once.
Sigmoid is computed as 0.5+0.5*tanh(x/2) (0.5 folded into Ww) so ScalarE only
needs the exp_and_others LUT set (tanh+exp): one table load, no thrash.
Softmax is batched across groups of 8 bags on partitions; its normalization
uses Exp's accum_out for the denominator. bias_rel == sum(softmax) == 1.0 is
used directly (error ~1e-9).
"""

import sys

if "/opt/trn_rl_repo" not in sys.path:
    sys.path.insert(0, "/opt/trn_rl_repo")

from contextlib import ExitStack

import numpy as np

import concourse.bass as bass
import concourse.tile as tile
from concourse import bacc, mybir

B, T, M, L = 256, 500, 512, 128
NCORES = 8
BSH = B // NCORES  # bags per core
TP = 125           # T-chunk (partition) size for natural layout; 4 chunks
NTC = T // TP      # 4
NMC = M // 128     # 4 M-chunks for transposed layout

F16 = mybir.dt.float16
F32 = mybir.dt.float32
AX = mybir.AxisListType
OP = mybir.AluOpType
AF = mybir.ActivationFunctionType


def build_bass(bsh=BSH, gb=8, stage=5):
    """Build the per-core Bass program. bsh bags per core, softmax-batched in
    groups of gb. stage<4 builds a prefix of the pipeline (debug bisection):
    1=logits only, 2=+softmax, 3=+a-transpose, 4=+pooling, 5=full."""
    assert bsh % gb == 0
    ng = bsh // gb
    nc = bacc.Bacc("TRN2", target_bir_lowering=False, debug=False)

    h_nat = nc.dram_tensor("h_nat", [bsh, T, M], F16, kind="ExternalInput").ap()
    h_tr = nc.dram_tensor("h_tr", [bsh, M, T], F16, kind="ExternalInput").ap()
    wvut = nc.dram_tensor("wvut", [M, 2 * L], F16, kind="ExternalInput").ap()
    wwt = nc.dram_tensor("wwt", [L, 1], F16, kind="ExternalInput").ap()
    bv_i = nc.dram_tensor("bv_i", [L, 1], F32, kind="ExternalInput").ap()
    buh_i = nc.dram_tensor("buh_i", [L, 1], F32, kind="ExternalInput").ap()
    bw_i = nc.dram_tensor("bw_i", [gb, 1], F32, kind="ExternalInput").ap()
    wc_i = nc.dram_tensor("wc_i", [bsh, 2, M], F32, kind="ExternalInput").ap()
    bc_i = nc.dram_tensor("bc_i", [bsh, 2], F32, kind="ExternalInput").ap()
    id_i = nc.dram_tensor("id_i", [gb, gb], F16, kind="ExternalInput").ap()

    out_o = nc.dram_tensor("out_o", [bsh, 2], F32, kind="ExternalOutput").ap()
    asf_o = nc.dram_tensor("asf_o", [bsh, T], F32, kind="ExternalOutput").ap()
    apre_o = nc.dram_tensor("apre_o", [bsh, T], F32, kind="ExternalOutput").ap()

    with tile.TileContext(nc) as tc, ExitStack() as ctx:
        consts = ctx.enter_context(tc.tile_pool(name="consts", bufs=1))
        htr_p = ctx.enter_context(tc.tile_pool(name="htr", bufs=6))
        hnat_p = ctx.enter_context(tc.tile_pool(name="hnat", bufs=6))
        act_p = ctx.enter_context(tc.tile_pool(name="act", bufs=3))
        grp_p = ctx.enter_context(tc.tile_pool(name="grp", bufs=2))
        mf_p = ctx.enter_context(tc.tile_pool(name="mf", bufs=1))
        ps_v = ctx.enter_context(tc.tile_pool(name="ps_v", bufs=2, space="PSUM"))
        ps_u = ctx.enter_context(tc.tile_pool(name="ps_u", bufs=2, space="PSUM"))
        ps_a = ctx.enter_context(tc.tile_pool(name="ps_a", bufs=2, space="PSUM"))
        ps_mf = ctx.enter_context(tc.tile_pool(name="ps_mf", bufs=1, space="PSUM"))
        ps_at = ctx.enter_context(tc.tile_pool(name="ps_at", bufs=1, space="PSUM"))

        # --- constants ---
        wvut_t = consts.tile([128, NMC, 2 * L], F16)
        nc.sync.dma_start(wvut_t, wvut.rearrange("(c p) l -> p c l", p=128))
        wwt_t = consts.tile([L, 1], F16)
        nc.sync.dma_start(wwt_t, wwt)
        bv_t = consts.tile([L, 1], F32)
        nc.sync.dma_start(bv_t, bv_i)
        buh_t = consts.tile([L, 1], F32)
        nc.sync.dma_start(buh_t, buh_i)
        bw_t = consts.tile([gb, 1], F32)
        nc.sync.dma_start(bw_t, bw_i)
        wc_t = consts.tile([bsh, 2, M], F32)
        nc.sync.dma_start(wc_t, wc_i)
        bc_t = consts.tile([bsh, 2], F32)
        nc.sync.dma_start(bc_t, bc_i)
        id_t = consts.tile([gb, gb], F16)
        nc.sync.dma_start(id_t, id_i)

        # ScalarE warmup. The ACT descriptor takes at most 2 sync waits and
        # the first use of a LUT set spends one on the table load, so: first
        # observe the const DMAs with plain copies (1 wait each), then
        # trigger the exp_and_others table load on inputs ScalarE itself
        # produced (no new waits). Exp first so walrus picks the set that
        # also contains tanh.
        warm = consts.tile([L, 1], F32)
        warm2 = consts.tile([L, 1], F32)
        nc.scalar.copy(warm, bv_t)
        nc.scalar.copy(warm2, buh_t)
        nc.scalar.activation(warm, warm2, AF.Exp)
        nc.scalar.activation(warm, warm2, AF.Tanh)

        mfeat_all = mf_p.tile([bsh, M], F32)

        for g in range(ng):
            apre_g = grp_p.tile([gb, T], F32, tag="apre_g")
            for j in range(gb):
                b = g * gb + j
                htr_t = htr_p.tile([128, NMC, T], F16, tag="htr")
                nc.sync.dma_start(htr_t, h_tr[b].rearrange("(c p) t -> p c t", p=128))
                vps = ps_v.tile([L, T], F32, tag="v")
                ups = ps_u.tile([L, T], F32, tag="u")
                for c in range(NMC):
                    nc.tensor.matmul(
                        vps, wvut_t[:, c, 0:L], htr_t[:, c, :],
                        start=(c == 0), stop=(c == NMC - 1),
                    )
                for c in range(NMC):
                    nc.tensor.matmul(
                        ups, wvut_t[:, c, L : 2 * L], htr_t[:, c, :],
                        start=(c == 0), stop=(c == NMC - 1),
                    )
                tv = act_p.tile([L, T], F16, tag="tv")
                nc.scalar.activation(tv, vps, AF.Tanh, bias=bv_t, scale=1.0)
                tu = act_p.tile([L, T], F16, tag="tu")
                nc.scalar.activation(tu, ups, AF.Tanh, bias=buh_t, scale=0.5)
                # VU' = (tu + 1) * tv  == 2 * V * sigmoid(U); the 1/2 lives in wwt
                vu = act_p.tile([L, T], F16, tag="vu")
                nc.vector.scalar_tensor_tensor(
                    vu, tu, 1.0, tv, op0=OP.add, op1=OP.mult
                )
                aps = ps_a.tile([1, T], F32, tag="apre")
                nc.tensor.matmul(aps, wwt_t, vu, start=True, stop=True)
                # bounce PSUM->SBUF (DMA can't read PSUM), then DMA shifts
                # the row onto partition j of the group tile
                aprow = act_p.tile([1, T], F32, tag="aprow")
                nc.vector.tensor_copy(aprow, aps)
                nc.gpsimd.dma_start(apre_g[j : j + 1, :], aprow)

            # --- group softmax over T (free dim), 8 bags on partitions ---
            apre_out = grp_p.tile([gb, T], F32, tag="apre_out")
            nc.vector.tensor_scalar_add(apre_out, apre_g, bw_t)
            nc.gpsimd.dma_start(apre_o[g * gb : (g + 1) * gb, :], apre_out)
            if stage < 2:
                continue
            negmax = grp_p.tile([gb, 1], F32, tag="negmax")
            nc.vector.tensor_reduce(
                negmax, apre_g, axis=AX.X, op=OP.max, negate=True
            )
            aexp = grp_p.tile([gb, T], F32, tag="aexp")
            zsum = grp_p.tile([gb, 1], F32, tag="zsum")
            nc.scalar.activation(
                aexp, apre_g, AF.Exp, bias=negmax, scale=1.0, accum_out=zsum
            )
            rz = grp_p.tile([gb, 1], F32, tag="rz")
            nc.vector.reciprocal(rz, zsum)
            asf_t = grp_p.tile([gb, T], F32, tag="asf")
            nc.vector.tensor_scalar_mul(asf_t, aexp, rz)
            nc.gpsimd.dma_start(asf_o[g * gb : (g + 1) * gb, :], asf_t)
            if stage < 3:
                continue
            an16 = grp_p.tile([gb, T], F16, tag="an16")
            nc.vector.tensor_scalar_mul(an16, aexp, rz)
            # aT[p, c, j] = a_norm[j, c*TP + p] via PE transpose of each T-chunk
            atps = ps_at.tile([TP, NTC, gb], F16, tag="atps")
            for c in range(NTC):
                nc.tensor.transpose(
                    atps[:, c, :], an16[:, c * TP : (c + 1) * TP], id_t
                )
            at_sb = grp_p.tile([TP, NTC, gb], F16, tag="at_sb")
            nc.vector.tensor_copy(at_sb, atps)
            if stage < 4:
                continue

            # --- pooling: Mfeat_b = sum_t a[t] * H[t, :] ---
            for j in range(gb):
                b = g * gb + j
                hnat_t = hnat_p.tile([TP, NTC, M], F16, tag="hnat")
                nc.sync.dma_start(
                    hnat_t, h_nat[b].rearrange("(c p) m -> p c m", p=TP)
                )
                mfps = ps_mf.tile([1, M], F32, tag="mf")
                for c in range(NTC):
                    nc.tensor.matmul(
                        mfps, at_sb[:, c, j : j + 1], hnat_t[:, c, :],
                        start=(c == 0), stop=(c == NTC - 1),
                    )
                mfrow = act_p.tile([1, M], F32, tag="mfrow")
                nc.vector.tensor_copy(mfrow, mfps)
                nc.gpsimd.dma_start(mfeat_all[b : b + 1, :], mfrow)

        # --- classifier: out = Mfeat @ Wc.T + bc ---
        if stage >= 5:
            prod = grp_p.tile([bsh, 2, M], F32, tag="prod")
            out_sb = grp_p.tile([bsh, 2], F32, tag="out_sb")
            for k in range(2):
                nc.vector.tensor_mul(prod[:, k, :], mfeat_all, wc_t[:, k, :])
            nc.vector.tensor_reduce(out_sb, prod, axis=AX.X, op=OP.add)
            out_fin = grp_p.tile([bsh, 2], F32, tag="out_fin")
            nc.vector.tensor_add(out_fin, out_sb, bc_t)
            nc.gpsimd.dma_start(out_o, out_fin)

    # Bacc legalizes sync waits (>=2 waits get split onto event-semaphore
    # instructions; matmul waits move to ldweights) -- required by walrus.
    nc.compile()
    return nc


def make_in_maps(H_cat, Wv, bv, Wu, bu, Ww, bw, Wc, bc, n_cores=NCORES, bsh=BSH, gb=8):
    """Host-side prep: shard bags, pre-transpose/cast H, pack weights."""
    H = np.ascontiguousarray(np.asarray(H_cat, dtype=np.float32))
    h16 = H.astype(np.float16)
    ht16 = np.ascontiguousarray(h16.transpose(0, 2, 1))
    wvut = np.ascontiguousarray(
        np.concatenate(
            [np.asarray(Wv, np.float32).T, np.asarray(Wu, np.float32).T], axis=1
        )
    ).astype(np.float16)
    wwt = (np.asarray(Ww, np.float32)[0] / 2.0).reshape(L, 1).astype(np.float16)
    bv_a = np.asarray(bv, np.float32).reshape(L, 1)
    buh = (np.asarray(bu, np.float32) / 2.0).reshape(L, 1)
    bw_a = np.full((gb, 1), np.float32(np.asarray(bw, np.float32)[0]), np.float32)
    wc_rep = np.ascontiguousarray(
        np.broadcast_to(np.asarray(Wc, np.float32)[None], (bsh, 2, M))
    )
    bc_rep = np.ascontiguousarray(
        np.broadcast_to(np.asarray(bc, np.float32)[None], (bsh, 2))
    )
    id8 = np.eye(gb, dtype=np.float16)
    in_maps = []
    for i in range(n_cores):
        sl = slice(i * bsh, (i + 1) * bsh)
        in_maps.append(
            {
                "h_nat": h16[sl],
                "h_tr": ht16[sl],
                "wvut": wvut,
                "wwt": wwt,
                "bv_i": bv_a,
                "buh_i": buh,
                "bw_i": bw_a,
                "wc_i": wc_rep,
                "bc_i": bc_rep,
                "id_i": id8,
            }
        )
    return in_maps


_NC_CACHE = {}


def get_nc(bsh=BSH, gb=8):
    key = (bsh, gb)
    if key not in _NC_CACHE:
        _NC_CACHE[key] = build_bass(bsh, gb)
    return _NC_CACHE[key]


def run(inputs, trace=False, **kwargs):
    from concourse.bass_utils import run_bass_kernel_spmd

    nc = get_nc()
    in_maps = make_in_maps(**inputs)
    res = run_bass_kernel_spmd(
        nc, in_maps, core_ids=list(range(NCORES)), trace=trace, **kwargs
    )
    outs = res.results
    out = np.concatenate([r["out_o"] for r in outs], axis=0)
    asf = np.concatenate([r["asf_o"] for r in outs], axis=0)
    apre = np.concatenate([r["apre_o"] for r in outs], axis=0).reshape(B, 1, T)
    return (out, asf, apre), res


def kernel(**inputs):
    (out, asf, apre), _ = run(inputs, trace=False)
    return out, asf, apre
